# revision 41
# baseline (speedup 1.0000x reference)
"""Single-query attention eval kernel for Trainium2, 8-core data parallel.

Problem (per full batch): enc_output [64, 2048, 1024] f32, h_n [64, 1024] f32.
  scores  = einsum('bqh,bsh->bqs', h_n[:, None, :], enc_output)
  attn    = softmax(scores, axis=-1)
  context = einsum('bqs,bsh->bqh', attn, enc_output)
  out     = concat([h_n[:, None, :], context], axis=2)   # [64, 1, 2048]

Sharding: pure data parallel, batch 64 -> 8 cores x 8 examples.

Per-core dataflow (memory-bound; enc shard read from HBM exactly once):
  - enc[b] streamed in 1 MiB chunks [128p, 2, 1024]; all 16 vtiles of one
    example stay SBUF-resident until its context matmuls consume them.
  - h_n broadcast to 128 partitions via ones-outer-product matmul (fp32r).
  - scores: fused multiply+row-reduce, split across two engines --
    DVE tensor_tensor_reduce (9 vtiles/example) and Pool/GPSIMD
    scalar_tensor_tensor with accum_out (7 vtiles/example).
  - softmax: DVE row-max, PE transpose + DVE max for the partition max,
    -max broadcast via ones-matmul, ACT Exp -> unnormalized weights w.
  - context: PE matmul in float32r (1 cyc/row vs 4 for fp32), lhsT = w
    column [128, 1], rhs = enc vtile halves [128, 512], accumulated over
    the 16 vtiles in PSUM (unnormalized).
  - Device outputs ctx_out [8, 1024] and w_out [128, 8*16]; the softmax
    denominator and the final concat/normalization happen host-side.
"""

import numpy as np

import concourse.mybir as mybir
import concourse.tile as tile
from concourse import bacc
from concourse.bass_utils import run_bass_kernel_spmd

B, S, H = 64, 2048, 1024
N_CORES = 8
B_LOC = B // N_CORES          # 8 examples per core

CHUNK_ROWS = 512              # s-rows per DMA (2 MiB per chunk)
J = CHUNK_ROWS // 128         # vtiles per chunk
N_CHUNKS = S // CHUNK_ROWS    # chunks per example
N_VT = S // 128               # 16 vtiles (columns of 128 scores) per example
ENC_BUFS = 18                 # 4.5 examples of chunks in flight (bf16)

FP32 = mybir.dt.float32
FP32R = mybir.dt.float32r
BF16 = mybir.dt.bfloat16


def build_nc():
    nc = bacc.Bacc(
        "TRN2",
        target_bir_lowering=False,
        debug=False,
        num_devices=N_CORES,
        num_swdge_queues=4,
    )
    enc = nc.dram_tensor("enc_output", [B_LOC, S, H], FP32, kind="ExternalInput").ap()
    hn = nc.dram_tensor("h_n", [B_LOC, H], FP32, kind="ExternalInput").ap()
    ident_dram = nc.dram_tensor("ident128", [128, 128], FP32, kind="ExternalInput").ap()
    ones_dram = nc.dram_tensor("ones128", [1, 128], FP32, kind="ExternalInput").ap()
    ctx_out = nc.dram_tensor("ctx_out", [B_LOC, H], FP32, kind="ExternalOutput").ap()
    lsum_out = nc.dram_tensor("lsum_out", [1, B_LOC], FP32, kind="ExternalOutput").ap()

    with tile.TileContext(nc) as tc:
        with (
            tc.tile_pool(name="const", bufs=1) as const_pool,
            tc.tile_pool(name="enc", bufs=ENC_BUFS) as enc_pool,
            tc.tile_pool(name="hnrow", bufs=2) as hnrow_pool,
            tc.tile_pool(name="hnb", bufs=3) as hnb_pool,
            tc.tile_pool(name="dead", bufs=1) as dead_pool,
            tc.tile_pool(name="prod", bufs=4) as prod_pool,
            tc.tile_pool(name="scorep", bufs=4) as score_pool,
            tc.tile_pool(name="wvp", bufs=4) as wv_pool,
            tc.tile_pool(name="sm", bufs=2) as sm_pool,
            tc.tile_pool(name="stage", bufs=2) as stage_pool,
            tc.tile_pool(name="ctx", bufs=4, space="PSUM") as ctx_pool,
            tc.tile_pool(name="psb", bufs=2, space="PSUM") as psb_pool,
            tc.tile_pool(name="pst", bufs=2, space="PSUM") as pst_pool,
        ):
            # Constants: identity for PE transpose (DMA'd in), +/-ones rows.
            ident = const_pool.tile([128, 128], FP32, tag="ident")
            nc.sync.dma_start(ident[:, :], ident_dram[:, :])
            neg_row = const_pool.tile([1, 128], FP32, tag="neg_row")
            nc.vector.memset(neg_row[:, :], -1.0)
            # fp32r ones row (DMA-produced so the fp32r matmul verifier rule
            # is satisfied) lets the hn broadcast run at 1 cyc/row.
            pos_row = const_pool.tile([1, 128], FP32R, tag="pos_row")
            nc.sync.dma_start(pos_row[:, :], ones_dram[:, :].bitcast(FP32R))
            ones_col = const_pool.tile([128, 1], FP32, tag="ones_col")
            nc.vector.memset(ones_col[:, :], 1.0)
            # Softmax denominators, one column per example; shipped once.
            lsums = const_pool.tile([1, B_LOC], FP32, tag="lsums")

            # Dead full-size output required by the fused DVE reduce op.
            dead_v = dead_pool.tile([128, H], BF16, tag="dead_v")

            # h_n row -> all 128 partitions: outer product with ones via PE,
            # then ACT copies PSUM -> SBUF.
            hn_bc = [None] * B_LOC

            def stage_hn(b):
                row = hnrow_pool.tile([1, H], FP32R, tag="hnrow", name=f"hnr{b}")
                nc.sync.dma_start(row[:, :], hn[b : b + 1, :].bitcast(FP32R))
                bc = hnb_pool.tile([128, H], BF16, tag="hnb", name=f"hnb{b}")
                for half in range(2):
                    fsl = slice(half * 512, (half + 1) * 512)
                    pb = psb_pool.tile(
                        [128, 512], FP32, tag="psb", name=f"psb{b}_{half}"
                    )
                    nc.tensor.matmul(pb[:, :], pos_row[:, :], row[:, fsl])
                    nc.scalar.copy(bc[:, fsl], pb[:, :])
                hn_bc[b] = bc

            stage_hn(0)
            stage_hn(1)

            for b in range(B_LOC):
                if b + 2 < B_LOC:
                    stage_hn(b + 2)

                # Per-example score/weight tiles (rotating) so one
                # example's softmax reads never serialize the next
                # example's score writes.
                scores = score_pool.tile([128, N_VT], FP32, tag="scores")
                w = wv_pool.tile([128, N_VT], BF16, tag="w")

                # Stream this example's enc chunks; fused mult+reduce scores
                # split across DVE and ACT.
                chunks = []
                for c in range(N_CHUNKS):
                    # bf16 chunks, cast on the fly by the gpsimd software
                    # DGE: halves SBUF footprint and doubles DVE throughput
                    # (2x perf mode); context matmuls run bf16 at 1 cyc/row.
                    ch = enc_pool.tile([128, J, H], BF16, tag="enc")
                    src = enc[b, c * CHUNK_ROWS : (c + 1) * CHUNK_ROWS, :].rearrange(
                        "(p j) h -> p j h", p=128
                    )
                    nc.gpsimd.dma_start(ch[:, :, :], src)
                    chunks.append(ch)
                    for j in range(J):
                        t = c * J + j
                        if t % 2 == 1:
                            # Fused product+row-reduce on DVE (no 2x mode
                            # for this opcode, ~1.22 us). Odd vtiles so the
                            # last vtile of an example takes the short path.
                            nc.vector.scalar_tensor_tensor(
                                out=dead_v[:, :],
                                in0=ch[:, j, :],
                                scalar=1.0,
                                in1=hn_bc[b][:, :],
                                op0=mybir.AluOpType.mult,
                                op1=mybir.AluOpType.mult,
                                accum_out=scores[:, t : t + 1],
                            )
                        else:
                            # bf16 tensor_tensor gets the DVE 2x perf mode
                            # (~0.61 us); the row-reduce rides ACT's slack.
                            prod = prod_pool.tile([128, H], BF16, tag="prod")
                            nc.vector.tensor_mul(
                                prod[:, :], ch[:, j, :], hn_bc[b][:, :]
                            )
                            nc.scalar.activation(
                                out=prod[:, :],
                                in_=prod[:, :],
                                func=mybir.ActivationFunctionType.Copy,
                                accum_out=scores[:, t : t + 1],
                            )

                # Max over the 2048 scores of example b (partition reduction
                # via PE transpose), then unnormalized exp weights.
                rmax = sm_pool.tile([128, 1], FP32, tag="rmax")
                nc.vector.reduce_max(
                    out=rmax[:, :], in_=scores[:, :], axis=mybir.AxisListType.X
                )
                rmax_t = pst_pool.tile([1, 128], FP32, tag="pst")
                nc.tensor.transpose(rmax_t[:, :], rmax[:, :], ident[:, :])
                mg = sm_pool.tile([1, 1], FP32, tag="mg")
                nc.vector.reduce_max(
                    out=mg[:, :], in_=rmax_t[:, :], axis=mybir.AxisListType.X
                )
                negm_ps = pst_pool.tile([128, 1], FP32, tag="pst")
                nc.tensor.matmul(negm_ps[:, :], neg_row[:, :], mg[:, :])
                negm = sm_pool.tile([128, 1], FP32, tag="negm")
                nc.scalar.copy(negm[:, :], negm_ps[:, :])

                nc.scalar.activation(
                    out=w[:, :],
                    in_=scores[:, :],
                    func=mybir.ActivationFunctionType.Exp,
                    bias=negm[:, 0:1],
                    scale=1.0,
                )

                # Context: PSUM-accumulated fp32r matmuls over the resident
                # chunks (unnormalized exp weights; 1/L applied host-side).
                ctx_half = [
                    ctx_pool.tile([1, 512], FP32, tag="ctx", name=f"ctx{b}_{i}")
                    for i in range(2)
                ]
                for c in range(N_CHUNKS):
                    for j in range(J):
                        t = c * J + j
                        for half in range(2):
                            nc.tensor.matmul(
                                ctx_half[half][:, :],
                                w[:, t : t + 1],
                                chunks[c][:, j, half * 512 : (half + 1) * 512],
                                start=(t == 0),
                                stop=(t == N_VT - 1),
                            )

                stage = stage_pool.tile([1, H], FP32, tag="stage")
                for half in range(2):
                    nc.scalar.copy(
                        stage[0:1, half * 512 : (half + 1) * 512],
                        ctx_half[half][:, :],
                    )
                nc.sync.dma_start(ctx_out[b : b + 1, :], stage[:, :])

                # Denominator on device: row-sum of w, then partition-sum
                # via ones-matmul into PSUM, parked in lsums column b.
                wsum = sm_pool.tile([128, 1], FP32, tag="wsum")
                nc.vector.tensor_reduce(
                    out=wsum[:, :], in_=w[:, :],
                    axis=mybir.AxisListType.X, op=mybir.AluOpType.add,
                )
                lsum_ps = pst_pool.tile([1, 1], FP32, tag="pst")
                nc.tensor.matmul(lsum_ps[:, :], wsum[:, :], ones_col[:, :])
                nc.scalar.copy(lsums[0:1, b : b + 1], lsum_ps[:, :])

            nc.sync.dma_start(lsum_out[:, :], lsums[:, :])

    nc.compile()
    return nc


_NC_CACHE = None


def _get_nc():
    global _NC_CACHE
    if _NC_CACHE is None:
        _NC_CACHE = build_nc()
    return _NC_CACHE


def kernel(enc_output: np.ndarray, h_n: np.ndarray) -> np.ndarray:
    enc_output = np.ascontiguousarray(enc_output, dtype=np.float32)
    h_n = np.ascontiguousarray(h_n, dtype=np.float32)
    assert enc_output.shape == (B, S, H)
    assert h_n.shape == (B, H)

    nc = _get_nc()
    ident = np.eye(128, dtype=np.float32)
    ones = np.ones((1, 128), dtype=np.float32)
    in_maps = [
        {
            "enc_output": enc_output[i * B_LOC : (i + 1) * B_LOC],
            "h_n": h_n[i * B_LOC : (i + 1) * B_LOC],
            "ident128": ident,
            "ones128": ones,
        }
        for i in range(N_CORES)
    ]
    res = run_bass_kernel_spmd(nc, in_maps, core_ids=list(range(N_CORES)))

    out = np.empty((B, 1, 2 * H), dtype=np.float32)
    for i in range(N_CORES):
        ctx = res.results[i]["ctx_out"]          # [B_LOC, H], unnormalized
        lsum = np.asarray(res.results[i]["lsum_out"], dtype=np.float64)[0]
        rows = slice(i * B_LOC, (i + 1) * B_LOC)
        out[rows, 0, :H] = h_n[rows]
        out[rows, 0, H:] = (ctx.astype(np.float64) / lsum[:, None]).astype(
            np.float32
        )
    return out


# revision 42
# speedup vs baseline: 1.0540x; 1.0540x over previous
"""Single-query attention eval kernel for Trainium2, 8-core data parallel.

Problem (per full batch): enc_output [64, 2048, 1024] f32, h_n [64, 1024] f32.
  scores  = einsum('bqh,bsh->bqs', h_n[:, None, :], enc_output)
  attn    = softmax(scores, axis=-1)
  context = einsum('bqs,bsh->bqh', attn, enc_output)
  out     = concat([h_n[:, None, :], context], axis=2)   # [64, 1, 2048]

Sharding: pure data parallel, batch 64 -> 8 cores x 8 examples.

Per-core dataflow (memory-bound; enc shard read from HBM exactly once):
  - enc[b] streamed in 1 MiB chunks [128p, 2, 1024]; all 16 vtiles of one
    example stay SBUF-resident until its context matmuls consume them.
  - h_n broadcast to 128 partitions via ones-outer-product matmul (fp32r).
  - scores: fused multiply+row-reduce, split across two engines --
    DVE tensor_tensor_reduce (9 vtiles/example) and Pool/GPSIMD
    scalar_tensor_tensor with accum_out (7 vtiles/example).
  - softmax: DVE row-max, PE transpose + DVE max for the partition max,
    -max broadcast via ones-matmul, ACT Exp -> unnormalized weights w.
  - context: PE matmul in float32r (1 cyc/row vs 4 for fp32), lhsT = w
    column [128, 1], rhs = enc vtile halves [128, 512], accumulated over
    the 16 vtiles in PSUM (unnormalized).
  - Device outputs ctx_out [8, 1024] and w_out [128, 8*16]; the softmax
    denominator and the final concat/normalization happen host-side.
"""

import numpy as np

import concourse.mybir as mybir
import concourse.tile as tile
from concourse import bacc
from concourse.bass_utils import run_bass_kernel_spmd

B, S, H = 64, 2048, 1024
N_CORES = 8
B_LOC = B // N_CORES          # 8 examples per core

CHUNK_ROWS = 512              # s-rows per DMA (2 MiB per chunk)
J = CHUNK_ROWS // 128         # vtiles per chunk
N_CHUNKS = S // CHUNK_ROWS    # chunks per example
N_VT = S // 128               # 16 vtiles (columns of 128 scores) per example
ENC_BUFS = 18                 # 4.5 examples of chunks in flight (bf16)

FP32 = mybir.dt.float32
FP32R = mybir.dt.float32r
BF16 = mybir.dt.bfloat16


def build_nc():
    nc = bacc.Bacc(
        "TRN2",
        target_bir_lowering=False,
        debug=False,
        num_devices=N_CORES,
        num_swdge_queues=4,
    )
    enc = nc.dram_tensor("enc_output", [B_LOC, S, H], FP32, kind="ExternalInput").ap()
    hn = nc.dram_tensor("h_n", [B_LOC, H], FP32, kind="ExternalInput").ap()
    ident_dram = nc.dram_tensor("ident128", [128, 128], FP32, kind="ExternalInput").ap()
    ones_dram = nc.dram_tensor("ones128", [1, 128], FP32, kind="ExternalInput").ap()
    ctx_out = nc.dram_tensor("ctx_out", [B_LOC, H], FP32, kind="ExternalOutput").ap()
    w_out = nc.dram_tensor(
        "w_out", [128, B_LOC * N_VT], BF16, kind="ExternalOutput"
    ).ap()

    with tile.TileContext(nc) as tc:
        with (
            tc.tile_pool(name="const", bufs=1) as const_pool,
            tc.tile_pool(name="enc", bufs=ENC_BUFS) as enc_pool,
            tc.tile_pool(name="hnrow", bufs=2) as hnrow_pool,
            tc.tile_pool(name="hnb", bufs=3) as hnb_pool,
            tc.tile_pool(name="dead", bufs=1) as dead_pool,
            tc.tile_pool(name="prod", bufs=4) as prod_pool,
            tc.tile_pool(name="scorep", bufs=4) as score_pool,
            tc.tile_pool(name="wvp", bufs=4) as wv_pool,
            tc.tile_pool(name="sm", bufs=2) as sm_pool,
            tc.tile_pool(name="stage", bufs=2) as stage_pool,
            tc.tile_pool(name="ctx", bufs=4, space="PSUM") as ctx_pool,
            tc.tile_pool(name="psb", bufs=2, space="PSUM") as psb_pool,
            tc.tile_pool(name="pst", bufs=2, space="PSUM") as pst_pool,
        ):
            # Constants: identity for PE transpose (DMA'd in), +/-ones rows.
            ident = const_pool.tile([128, 128], FP32, tag="ident")
            nc.sync.dma_start(ident[:, :], ident_dram[:, :])
            neg_row = const_pool.tile([1, 128], FP32, tag="neg_row")
            nc.vector.memset(neg_row[:, :], -1.0)
            # fp32r ones row (DMA-produced so the fp32r matmul verifier rule
            # is satisfied) lets the hn broadcast run at 1 cyc/row.
            pos_row = const_pool.tile([1, 128], FP32R, tag="pos_row")
            nc.sync.dma_start(pos_row[:, :], ones_dram[:, :].bitcast(FP32R))

            # Dead full-size output required by the fused DVE reduce op.
            dead_v = dead_pool.tile([128, H], BF16, tag="dead_v")

            # h_n row -> all 128 partitions: outer product with ones via PE,
            # then ACT copies PSUM -> SBUF.
            hn_bc = [None] * B_LOC

            def stage_hn(b):
                row = hnrow_pool.tile([1, H], FP32R, tag="hnrow", name=f"hnr{b}")
                nc.sync.dma_start(row[:, :], hn[b : b + 1, :].bitcast(FP32R))
                bc = hnb_pool.tile([128, H], BF16, tag="hnb", name=f"hnb{b}")
                for half in range(2):
                    fsl = slice(half * 512, (half + 1) * 512)
                    pb = psb_pool.tile(
                        [128, 512], FP32, tag="psb", name=f"psb{b}_{half}"
                    )
                    nc.tensor.matmul(pb[:, :], pos_row[:, :], row[:, fsl])
                    nc.scalar.copy(bc[:, fsl], pb[:, :])
                hn_bc[b] = bc

            stage_hn(0)
            stage_hn(1)

            for b in range(B_LOC):
                if b + 2 < B_LOC:
                    stage_hn(b + 2)

                # Per-example score/weight tiles (rotating) so one
                # example's softmax reads never serialize the next
                # example's score writes.
                scores = score_pool.tile([128, N_VT], FP32, tag="scores")
                w = wv_pool.tile([128, N_VT], BF16, tag="w")

                # Stream this example's enc chunks; fused mult+reduce scores
                # split across DVE and ACT.
                chunks = []
                for c in range(N_CHUNKS):
                    # bf16 chunks, cast on the fly by the gpsimd software
                    # DGE: halves SBUF footprint and doubles DVE throughput
                    # (2x perf mode); context matmuls run bf16 at 1 cyc/row.
                    ch = enc_pool.tile([128, J, H], BF16, tag="enc")
                    src = enc[b, c * CHUNK_ROWS : (c + 1) * CHUNK_ROWS, :].rearrange(
                        "(p j) h -> p j h", p=128
                    )
                    nc.gpsimd.dma_start(ch[:, :, :], src)
                    chunks.append(ch)
                    for j in range(J):
                        t = c * J + j
                        if t % 2 == 1:
                            # Fused product+row-reduce on DVE (no 2x mode
                            # for this opcode, ~1.22 us). Odd vtiles so the
                            # last vtile of an example takes the short path.
                            nc.vector.scalar_tensor_tensor(
                                out=dead_v[:, :],
                                in0=ch[:, j, :],
                                scalar=1.0,
                                in1=hn_bc[b][:, :],
                                op0=mybir.AluOpType.mult,
                                op1=mybir.AluOpType.mult,
                                accum_out=scores[:, t : t + 1],
                            )
                        else:
                            # bf16 tensor_tensor gets the DVE 2x perf mode
                            # (~0.61 us); the row-reduce rides ACT's slack.
                            prod = prod_pool.tile([128, H], BF16, tag="prod")
                            nc.vector.tensor_mul(
                                prod[:, :], ch[:, j, :], hn_bc[b][:, :]
                            )
                            nc.scalar.activation(
                                out=prod[:, :],
                                in_=prod[:, :],
                                func=mybir.ActivationFunctionType.Copy,
                                accum_out=scores[:, t : t + 1],
                            )

                # Max over the 2048 scores of example b (partition reduction
                # via PE transpose), then unnormalized exp weights.
                rmax = sm_pool.tile([128, 1], FP32, tag="rmax")
                nc.vector.reduce_max(
                    out=rmax[:, :], in_=scores[:, :], axis=mybir.AxisListType.X
                )
                rmax_t = pst_pool.tile([1, 128], FP32, tag="pst")
                nc.tensor.transpose(rmax_t[:, :], rmax[:, :], ident[:, :])
                mg = sm_pool.tile([1, 1], FP32, tag="mg")
                nc.vector.reduce_max(
                    out=mg[:, :], in_=rmax_t[:, :], axis=mybir.AxisListType.X
                )
                negm_ps = pst_pool.tile([128, 1], FP32, tag="pst")
                nc.tensor.matmul(negm_ps[:, :], neg_row[:, :], mg[:, :])
                negm = sm_pool.tile([128, 1], FP32, tag="negm")
                nc.scalar.copy(negm[:, :], negm_ps[:, :])

                nc.scalar.activation(
                    out=w[:, :],
                    in_=scores[:, :],
                    func=mybir.ActivationFunctionType.Exp,
                    bias=negm[:, 0:1],
                    scale=1.0,
                )

                # Context: PSUM-accumulated fp32r matmuls over the resident
                # chunks (unnormalized exp weights; 1/L applied host-side).
                ctx_half = [
                    ctx_pool.tile([1, 512], FP32, tag="ctx", name=f"ctx{b}_{i}")
                    for i in range(2)
                ]
                for c in range(N_CHUNKS):
                    for j in range(J):
                        t = c * J + j
                        for half in range(2):
                            nc.tensor.matmul(
                                ctx_half[half][:, :],
                                w[:, t : t + 1],
                                chunks[c][:, j, half * 512 : (half + 1) * 512],
                                start=(t == 0),
                                stop=(t == N_VT - 1),
                            )

                stage = stage_pool.tile([1, H], FP32, tag="stage")
                for half in range(2):
                    nc.scalar.copy(
                        stage[0:1, half * 512 : (half + 1) * 512],
                        ctx_half[half][:, :],
                    )
                nc.sync.dma_start(ctx_out[b : b + 1, :], stage[:, :])
                nc.sync.dma_start(w_out[:, b * N_VT : (b + 1) * N_VT], w[:, :])

    nc.compile()
    return nc


_NC_CACHE = None


def _get_nc():
    global _NC_CACHE
    if _NC_CACHE is None:
        _NC_CACHE = build_nc()
    return _NC_CACHE


def kernel(enc_output: np.ndarray, h_n: np.ndarray) -> np.ndarray:
    enc_output = np.ascontiguousarray(enc_output, dtype=np.float32)
    h_n = np.ascontiguousarray(h_n, dtype=np.float32)
    assert enc_output.shape == (B, S, H)
    assert h_n.shape == (B, H)

    nc = _get_nc()
    ident = np.eye(128, dtype=np.float32)
    ones = np.ones((1, 128), dtype=np.float32)
    in_maps = [
        {
            "enc_output": enc_output[i * B_LOC : (i + 1) * B_LOC],
            "h_n": h_n[i * B_LOC : (i + 1) * B_LOC],
            "ident128": ident,
            "ones128": ones,
        }
        for i in range(N_CORES)
    ]
    res = run_bass_kernel_spmd(nc, in_maps, core_ids=list(range(N_CORES)))

    out = np.empty((B, 1, 2 * H), dtype=np.float32)
    for i in range(N_CORES):
        ctx = res.results[i]["ctx_out"]          # [B_LOC, H], unnormalized
        wv = res.results[i]["w_out"]             # [128, B_LOC * N_VT]
        lsum = (
            wv.astype(np.float64)
            .reshape(128, B_LOC, N_VT)
            .sum(axis=(0, 2))
        )                                        # [B_LOC]
        rows = slice(i * B_LOC, (i + 1) * B_LOC)
        out[rows, 0, :H] = h_n[rows]
        out[rows, 0, H:] = (ctx.astype(np.float64) / lsum[:, None]).astype(
            np.float32
        )
    return out


# revision 43
# speedup vs baseline: 1.1405x; 1.0821x over previous
"""Single-query attention eval kernel for Trainium2, 8-core data parallel.

Problem (per full batch): enc_output [64, 2048, 1024] f32, h_n [64, 1024] f32.
  scores  = einsum('bqh,bsh->bqs', h_n[:, None, :], enc_output)
  attn    = softmax(scores, axis=-1)
  context = einsum('bqs,bsh->bqh', attn, enc_output)
  out     = concat([h_n[:, None, :], context], axis=2)   # [64, 1, 2048]

Sharding: pure data parallel, batch 64 -> 8 cores x 8 examples.

Per-core dataflow (memory-bound; enc shard read from HBM exactly once):
  - enc[b] streamed in 1 MiB chunks [128p, 2, 1024]; all 16 vtiles of one
    example stay SBUF-resident until its context matmuls consume them.
  - h_n broadcast to 128 partitions via ones-outer-product matmul (fp32r).
  - scores: fused multiply+row-reduce, split across two engines --
    DVE tensor_tensor_reduce (9 vtiles/example) and Pool/GPSIMD
    scalar_tensor_tensor with accum_out (7 vtiles/example).
  - softmax: DVE row-max, PE transpose + DVE max for the partition max,
    -max broadcast via ones-matmul, ACT Exp -> unnormalized weights w.
  - context: PE matmul in float32r (1 cyc/row vs 4 for fp32), lhsT = w
    column [128, 1], rhs = enc vtile halves [128, 512], accumulated over
    the 16 vtiles in PSUM (unnormalized).
  - Device outputs ctx_out [8, 1024] and w_out [128, 8*16]; the softmax
    denominator and the final concat/normalization happen host-side.
"""

import numpy as np

import concourse.mybir as mybir
import concourse.tile as tile
from concourse import bacc
from concourse.bass_utils import run_bass_kernel_spmd

B, S, H = 64, 2048, 1024
N_CORES = 8
B_LOC = B // N_CORES          # 8 examples per core

CHUNK_ROWS = 512              # s-rows per DMA (2 MiB per chunk)
J = CHUNK_ROWS // 128         # vtiles per chunk
N_CHUNKS = S // CHUNK_ROWS    # chunks per example
N_VT = S // 128               # 16 vtiles (columns of 128 scores) per example
ENC_BUFS = 20                 # 5 examples of chunks in flight (bf16)

FP32 = mybir.dt.float32
FP32R = mybir.dt.float32r
BF16 = mybir.dt.bfloat16


def build_nc():
    nc = bacc.Bacc(
        "TRN2",
        target_bir_lowering=False,
        debug=False,
        num_devices=N_CORES,
        num_swdge_queues=4,
    )
    enc = nc.dram_tensor("enc_output", [B_LOC, S, H], FP32, kind="ExternalInput").ap()
    hn = nc.dram_tensor("h_n", [B_LOC, H], FP32, kind="ExternalInput").ap()
    ident_dram = nc.dram_tensor("ident128", [128, 128], FP32, kind="ExternalInput").ap()
    ones_dram = nc.dram_tensor("ones128", [1, 128], FP32, kind="ExternalInput").ap()
    ctx_out = nc.dram_tensor("ctx_out", [B_LOC, H], FP32, kind="ExternalOutput").ap()
    w_out = nc.dram_tensor(
        "w_out", [128, B_LOC * N_VT], BF16, kind="ExternalOutput"
    ).ap()

    with tile.TileContext(nc) as tc:
        with (
            tc.tile_pool(name="const", bufs=1) as const_pool,
            tc.tile_pool(name="enc", bufs=ENC_BUFS) as enc_pool,
            tc.tile_pool(name="hnrow", bufs=2) as hnrow_pool,
            tc.tile_pool(name="hnb", bufs=3) as hnb_pool,
            tc.tile_pool(name="dead", bufs=1) as dead_pool,
            tc.tile_pool(name="prod", bufs=6) as prod_pool,
            tc.tile_pool(name="scorep", bufs=4) as score_pool,
            tc.tile_pool(name="wvp", bufs=4) as wv_pool,
            tc.tile_pool(name="sm", bufs=2) as sm_pool,
            tc.tile_pool(name="stage", bufs=2) as stage_pool,
            tc.tile_pool(name="ctx", bufs=4, space="PSUM") as ctx_pool,
            tc.tile_pool(name="psb", bufs=1, space="PSUM") as psb_pool,
            tc.tile_pool(name="pst", bufs=3, space="PSUM") as pst_pool,
        ):
            # Constants: identity for PE transpose (DMA'd in), +/-ones rows.
            ident = const_pool.tile([128, 128], FP32, tag="ident")
            nc.sync.dma_start(ident[:, :], ident_dram[:, :])
            neg_row = const_pool.tile([1, 128], FP32, tag="neg_row")
            nc.vector.memset(neg_row[:, :], -1.0)
            # fp32r ones row (DMA-produced so the fp32r matmul verifier rule
            # is satisfied) lets the hn broadcast run at 1 cyc/row.
            pos_row = const_pool.tile([1, 128], FP32R, tag="pos_row")
            nc.sync.dma_start(pos_row[:, :], ones_dram[:, :].bitcast(FP32R))

            # Dead full-size output required by the fused DVE reduce op.
            dead_v = dead_pool.tile([128, H], BF16, tag="dead_v")

            # h_n row -> all 128 partitions: outer product with ones via PE,
            # then ACT copies PSUM -> SBUF.
            hn_bc = [None] * B_LOC

            def stage_hn(b):
                row = hnrow_pool.tile([1, H], FP32R, tag="hnrow", name=f"hnr{b}")
                nc.sync.dma_start(row[:, :], hn[b : b + 1, :].bitcast(FP32R))
                bc = hnb_pool.tile([128, H], BF16, tag="hnb", name=f"hnb{b}")
                for half in range(2):
                    fsl = slice(half * 512, (half + 1) * 512)
                    pb = psb_pool.tile(
                        [128, 512], FP32, tag="psb", name=f"psb{b}_{half}"
                    )
                    nc.tensor.matmul(pb[:, :], pos_row[:, :], row[:, fsl])
                    nc.scalar.copy(bc[:, fsl], pb[:, :])
                hn_bc[b] = bc

            stage_hn(0)
            stage_hn(1)

            for b in range(B_LOC):
                if b + 2 < B_LOC:
                    stage_hn(b + 2)

                # Per-example score/weight tiles (rotating) so one
                # example's softmax reads never serialize the next
                # example's score writes.
                scores = score_pool.tile([128, N_VT], FP32, tag="scores")
                w = wv_pool.tile([128, N_VT], BF16, tag="w")

                # Stream this example's enc chunks; fused mult+reduce scores
                # split across DVE and ACT.
                chunks = []
                for c in range(N_CHUNKS):
                    # bf16 chunks, cast on the fly by the gpsimd software
                    # DGE: halves SBUF footprint and doubles DVE throughput
                    # (2x perf mode); context matmuls run bf16 at 1 cyc/row.
                    ch = enc_pool.tile([128, J, H], BF16, tag="enc")
                    src = enc[b, c * CHUNK_ROWS : (c + 1) * CHUNK_ROWS, :].rearrange(
                        "(p j) h -> p j h", p=128
                    )
                    nc.gpsimd.dma_start(ch[:, :, :], src)
                    chunks.append(ch)
                    for j in range(J):
                        t = c * J + j
                        if t % 2 == 1:
                            # Fused product+row-reduce on DVE (no 2x mode
                            # for this opcode, ~1.22 us). Odd vtiles so the
                            # last vtile of an example takes the short path.
                            nc.vector.scalar_tensor_tensor(
                                out=dead_v[:, :],
                                in0=ch[:, j, :],
                                scalar=1.0,
                                in1=hn_bc[b][:, :],
                                op0=mybir.AluOpType.mult,
                                op1=mybir.AluOpType.mult,
                                accum_out=scores[:, t : t + 1],
                            )
                        else:
                            # bf16 tensor_tensor gets the DVE 2x perf mode
                            # (~0.61 us); the row-reduce rides ACT's slack.
                            prod = prod_pool.tile([128, H], BF16, tag="prod")
                            nc.vector.tensor_mul(
                                prod[:, :], ch[:, j, :], hn_bc[b][:, :]
                            )
                            nc.scalar.activation(
                                out=prod[:, :],
                                in_=prod[:, :],
                                func=mybir.ActivationFunctionType.Copy,
                                accum_out=scores[:, t : t + 1],
                            )

                # Max over the 2048 scores of example b (partition reduction
                # via PE transpose), then unnormalized exp weights.
                rmax = sm_pool.tile([128, 1], FP32, tag="rmax")
                nc.vector.reduce_max(
                    out=rmax[:, :], in_=scores[:, :], axis=mybir.AxisListType.X
                )
                rmax_t = pst_pool.tile([1, 128], FP32, tag="pst")
                nc.tensor.transpose(rmax_t[:, :], rmax[:, :], ident[:, :])
                mg = sm_pool.tile([1, 1], FP32, tag="mg")
                nc.vector.reduce_max(
                    out=mg[:, :], in_=rmax_t[:, :], axis=mybir.AxisListType.X
                )
                negm_ps = pst_pool.tile([128, 1], FP32, tag="pst")
                nc.tensor.matmul(negm_ps[:, :], neg_row[:, :], mg[:, :])
                negm = sm_pool.tile([128, 1], FP32, tag="negm")
                nc.scalar.copy(negm[:, :], negm_ps[:, :])

                nc.scalar.activation(
                    out=w[:, :],
                    in_=scores[:, :],
                    func=mybir.ActivationFunctionType.Exp,
                    bias=negm[:, 0:1],
                    scale=1.0,
                )

                # Context: PSUM-accumulated fp32r matmuls over the resident
                # chunks (unnormalized exp weights; 1/L applied host-side).
                ctx_half = [
                    ctx_pool.tile([1, 512], FP32, tag="ctx", name=f"ctx{b}_{i}")
                    for i in range(2)
                ]
                for c in range(N_CHUNKS):
                    for j in range(J):
                        t = c * J + j
                        for half in range(2):
                            nc.tensor.matmul(
                                ctx_half[half][:, :],
                                w[:, t : t + 1],
                                chunks[c][:, j, half * 512 : (half + 1) * 512],
                                start=(t == 0),
                                stop=(t == N_VT - 1),
                            )

                stage = stage_pool.tile([1, H], FP32, tag="stage")
                for half in range(2):
                    nc.scalar.copy(
                        stage[0:1, half * 512 : (half + 1) * 512],
                        ctx_half[half][:, :],
                    )
                nc.sync.dma_start(ctx_out[b : b + 1, :], stage[:, :])
                nc.sync.dma_start(w_out[:, b * N_VT : (b + 1) * N_VT], w[:, :])

    nc.compile()
    return nc


_NC_CACHE = None


def _get_nc():
    global _NC_CACHE
    if _NC_CACHE is None:
        _NC_CACHE = build_nc()
    return _NC_CACHE


def kernel(enc_output: np.ndarray, h_n: np.ndarray) -> np.ndarray:
    enc_output = np.ascontiguousarray(enc_output, dtype=np.float32)
    h_n = np.ascontiguousarray(h_n, dtype=np.float32)
    assert enc_output.shape == (B, S, H)
    assert h_n.shape == (B, H)

    nc = _get_nc()
    ident = np.eye(128, dtype=np.float32)
    ones = np.ones((1, 128), dtype=np.float32)
    in_maps = [
        {
            "enc_output": enc_output[i * B_LOC : (i + 1) * B_LOC],
            "h_n": h_n[i * B_LOC : (i + 1) * B_LOC],
            "ident128": ident,
            "ones128": ones,
        }
        for i in range(N_CORES)
    ]
    res = run_bass_kernel_spmd(nc, in_maps, core_ids=list(range(N_CORES)))

    out = np.empty((B, 1, 2 * H), dtype=np.float32)
    for i in range(N_CORES):
        ctx = res.results[i]["ctx_out"]          # [B_LOC, H], unnormalized
        wv = res.results[i]["w_out"]             # [128, B_LOC * N_VT]
        lsum = (
            wv.astype(np.float64)
            .reshape(128, B_LOC, N_VT)
            .sum(axis=(0, 2))
        )                                        # [B_LOC]
        rows = slice(i * B_LOC, (i + 1) * B_LOC)
        out[rows, 0, :H] = h_n[rows]
        out[rows, 0, H:] = (ctx.astype(np.float64) / lsum[:, None]).astype(
            np.float32
        )
    return out


# revision 44
# speedup vs baseline: 1.1414x; 1.0008x over previous
"""Single-query attention eval kernel for Trainium2, 8-core data parallel.

Problem (per full batch): enc_output [64, 2048, 1024] f32, h_n [64, 1024] f32.
  scores  = einsum('bqh,bsh->bqs', h_n[:, None, :], enc_output)
  attn    = softmax(scores, axis=-1)
  context = einsum('bqs,bsh->bqh', attn, enc_output)
  out     = concat([h_n[:, None, :], context], axis=2)   # [64, 1, 2048]

Sharding: pure data parallel, batch 64 -> 8 cores x 8 examples.

Per-core dataflow (memory-bound; enc shard read from HBM exactly once,
cast to bf16 in flight):
  - enc[b] streamed in 2 MiB chunks [128p, 4, 1024] through the gpsimd
    software DGE, which casts f32 -> bf16 on the way into SBUF; all 16
    vtiles of one example stay resident until its context matmuls finish.
  - h_n broadcast to 128 partitions via fp32r ones-outer-product matmul,
    parked as bf16.
  - scores: per vtile, alternating DVE scalar_tensor_tensor (fused
    product+row-reduce, no DVE perf mode) and DVE tensor_mul (bf16 2x
    perf mode) with the row-reduce on ACT's accumulate-copy path.
  - softmax: DVE row-max, PE transpose + DVE max for the partition max,
    -max broadcast via ones-matmul, ACT Exp -> unnormalized bf16 w.
    Per-example rotating score/w tiles avoid cross-example WAR stalls.
  - context: bf16 PE matmuls (1 cyc/row), lhsT = w column [128, 1], rhs =
    enc vtile halves [128, 512], accumulated over 16 vtiles in PSUM.
  - Device outputs ctx_out [8, 1024] (unnormalized) and w_out (bf16); the
    softmax denominator and concat/normalization happen host-side in f64.
"""

import numpy as np

import concourse.mybir as mybir
import concourse.tile as tile
from concourse import bacc
from concourse.bass_utils import run_bass_kernel_spmd

B, S, H = 64, 2048, 1024
N_CORES = 8
B_LOC = B // N_CORES          # 8 examples per core

CHUNK_ROWS = 512              # s-rows per DMA (2 MiB per chunk)
J = CHUNK_ROWS // 128         # vtiles per chunk
N_CHUNKS = S // CHUNK_ROWS    # chunks per example
N_VT = S // 128               # 16 vtiles (columns of 128 scores) per example
ENC_BUFS = 20                 # 5 examples of chunks in flight (bf16)

FP32 = mybir.dt.float32
FP32R = mybir.dt.float32r
BF16 = mybir.dt.bfloat16


def build_nc():
    nc = bacc.Bacc(
        "TRN2",
        target_bir_lowering=False,
        debug=False,
        num_devices=N_CORES,
        num_swdge_queues=4,
    )
    enc = nc.dram_tensor("enc_output", [B_LOC, S, H], FP32, kind="ExternalInput").ap()
    hn = nc.dram_tensor("h_n", [B_LOC, H], FP32, kind="ExternalInput").ap()
    ident_dram = nc.dram_tensor("ident128", [128, 128], FP32, kind="ExternalInput").ap()
    ones_dram = nc.dram_tensor("ones128", [1, 128], FP32, kind="ExternalInput").ap()
    ctx_out = nc.dram_tensor("ctx_out", [B_LOC, H], FP32, kind="ExternalOutput").ap()
    w_out = nc.dram_tensor(
        "w_out", [128, B_LOC * N_VT], BF16, kind="ExternalOutput"
    ).ap()

    with tile.TileContext(nc) as tc:
        with (
            tc.tile_pool(name="const", bufs=1) as const_pool,
            tc.tile_pool(name="enc", bufs=ENC_BUFS) as enc_pool,
            tc.tile_pool(name="hnrow", bufs=2) as hnrow_pool,
            tc.tile_pool(name="hnb", bufs=3) as hnb_pool,
            tc.tile_pool(name="dead", bufs=1) as dead_pool,
            tc.tile_pool(name="prod", bufs=6) as prod_pool,
            tc.tile_pool(name="scorep", bufs=4) as score_pool,
            tc.tile_pool(name="wvp", bufs=4) as wv_pool,
            tc.tile_pool(name="sm", bufs=2) as sm_pool,
            tc.tile_pool(name="stage", bufs=2) as stage_pool,
            tc.tile_pool(name="ctx", bufs=4, space="PSUM") as ctx_pool,
            tc.tile_pool(name="psb", bufs=1, space="PSUM") as psb_pool,
            tc.tile_pool(name="pst", bufs=3, space="PSUM") as pst_pool,
        ):
            # Constants: identity for PE transpose (DMA'd in), +/-ones rows.
            ident = const_pool.tile([128, 128], FP32, tag="ident")
            nc.sync.dma_start(ident[:, :], ident_dram[:, :])
            neg_row = const_pool.tile([1, 128], FP32, tag="neg_row")
            nc.vector.memset(neg_row[:, :], -1.0)
            # fp32r ones row (DMA-produced so the fp32r matmul verifier rule
            # is satisfied) lets the hn broadcast run at 1 cyc/row.
            pos_row = const_pool.tile([1, 128], FP32R, tag="pos_row")
            nc.sync.dma_start(pos_row[:, :], ones_dram[:, :].bitcast(FP32R))

            # Dead full-size output required by the fused DVE reduce op.
            dead_v = dead_pool.tile([128, H], BF16, tag="dead_v")

            # h_n row -> all 128 partitions: outer product with ones via PE,
            # then ACT copies PSUM -> SBUF.
            hn_bc = [None] * B_LOC

            def stage_hn(b):
                row = hnrow_pool.tile([1, H], FP32R, tag="hnrow", name=f"hnr{b}")
                nc.sync.dma_start(row[:, :], hn[b : b + 1, :].bitcast(FP32R))
                bc = hnb_pool.tile([128, H], BF16, tag="hnb", name=f"hnb{b}")
                for half in range(2):
                    fsl = slice(half * 512, (half + 1) * 512)
                    pb = psb_pool.tile(
                        [128, 512], FP32, tag="psb", name=f"psb{b}_{half}"
                    )
                    nc.tensor.matmul(pb[:, :], pos_row[:, :], row[:, fsl])
                    nc.scalar.copy(bc[:, fsl], pb[:, :])
                hn_bc[b] = bc

            stage_hn(0)
            stage_hn(1)

            for b in range(B_LOC):
                if b + 2 < B_LOC:
                    stage_hn(b + 2)

                # Per-example score/weight tiles (rotating) so one
                # example's softmax reads never serialize the next
                # example's score writes.
                scores = score_pool.tile([128, N_VT], FP32, tag="scores")
                w = wv_pool.tile([128, N_VT], BF16, tag="w")

                # Stream this example's enc chunks; fused mult+reduce scores
                # split across DVE and ACT.
                chunks = []
                for c in range(N_CHUNKS):
                    # bf16 chunks, cast on the fly by the gpsimd software
                    # DGE: halves SBUF footprint and doubles DVE throughput
                    # (2x perf mode); context matmuls run bf16 at 1 cyc/row.
                    ch = enc_pool.tile([128, J, H], BF16, tag="enc")
                    src = enc[b, c * CHUNK_ROWS : (c + 1) * CHUNK_ROWS, :].rearrange(
                        "(p j) h -> p j h", p=128
                    )
                    nc.gpsimd.dma_start(ch[:, :, :], src)
                    chunks.append(ch)
                    for j in range(J):
                        t = c * J + j
                        if t % 2 == 1:
                            # Fused product+row-reduce on DVE (no 2x mode
                            # for this opcode, ~1.22 us). Odd vtiles so the
                            # last vtile of an example takes the short path.
                            nc.vector.scalar_tensor_tensor(
                                out=dead_v[:, :],
                                in0=ch[:, j, :],
                                scalar=1.0,
                                in1=hn_bc[b][:, :],
                                op0=mybir.AluOpType.mult,
                                op1=mybir.AluOpType.mult,
                                accum_out=scores[:, t : t + 1],
                            )
                        else:
                            # bf16 tensor_tensor gets the DVE 2x perf mode
                            # (~0.61 us); the row-reduce rides ACT's slack.
                            prod = prod_pool.tile([128, H], BF16, tag="prod")
                            nc.vector.tensor_mul(
                                prod[:, :], ch[:, j, :], hn_bc[b][:, :]
                            )
                            nc.scalar.activation(
                                out=prod[:, :],
                                in_=prod[:, :],
                                func=mybir.ActivationFunctionType.Copy,
                                accum_out=scores[:, t : t + 1],
                            )

                # Max over the 2048 scores of example b (partition reduction
                # via PE transpose), then unnormalized exp weights.
                rmax = sm_pool.tile([128, 1], FP32, tag="rmax")
                nc.vector.reduce_max(
                    out=rmax[:, :], in_=scores[:, :], axis=mybir.AxisListType.X
                )
                rmax_t = pst_pool.tile([1, 128], FP32, tag="pst")
                nc.tensor.transpose(rmax_t[:, :], rmax[:, :], ident[:, :])
                mg = sm_pool.tile([1, 1], FP32, tag="mg")
                nc.vector.reduce_max(
                    out=mg[:, :], in_=rmax_t[:, :], axis=mybir.AxisListType.X
                )
                negm_ps = pst_pool.tile([128, 1], FP32, tag="pst")
                nc.tensor.matmul(negm_ps[:, :], neg_row[:, :], mg[:, :])
                negm = sm_pool.tile([128, 1], FP32, tag="negm")
                nc.scalar.copy(negm[:, :], negm_ps[:, :])

                nc.scalar.activation(
                    out=w[:, :],
                    in_=scores[:, :],
                    func=mybir.ActivationFunctionType.Exp,
                    bias=negm[:, 0:1],
                    scale=1.0,
                )

                # Context: PSUM-accumulated fp32r matmuls over the resident
                # chunks (unnormalized exp weights; 1/L applied host-side).
                ctx_half = [
                    ctx_pool.tile([1, 512], FP32, tag="ctx", name=f"ctx{b}_{i}")
                    for i in range(2)
                ]
                for c in range(N_CHUNKS):
                    for j in range(J):
                        t = c * J + j
                        for half in range(2):
                            nc.tensor.matmul(
                                ctx_half[half][:, :],
                                w[:, t : t + 1],
                                chunks[c][:, j, half * 512 : (half + 1) * 512],
                                start=(t == 0),
                                stop=(t == N_VT - 1),
                            )

                stage = stage_pool.tile([1, H], FP32, tag="stage")
                for half in range(2):
                    nc.scalar.copy(
                        stage[0:1, half * 512 : (half + 1) * 512],
                        ctx_half[half][:, :],
                    )
                nc.sync.dma_start(ctx_out[b : b + 1, :], stage[:, :])
                nc.sync.dma_start(w_out[:, b * N_VT : (b + 1) * N_VT], w[:, :])

    nc.compile()
    return nc


_NC_CACHE = None


def _get_nc():
    global _NC_CACHE
    if _NC_CACHE is None:
        _NC_CACHE = build_nc()
    return _NC_CACHE


def kernel(enc_output: np.ndarray, h_n: np.ndarray) -> np.ndarray:
    enc_output = np.ascontiguousarray(enc_output, dtype=np.float32)
    h_n = np.ascontiguousarray(h_n, dtype=np.float32)
    assert enc_output.shape == (B, S, H)
    assert h_n.shape == (B, H)

    nc = _get_nc()
    ident = np.eye(128, dtype=np.float32)
    ones = np.ones((1, 128), dtype=np.float32)
    in_maps = [
        {
            "enc_output": enc_output[i * B_LOC : (i + 1) * B_LOC],
            "h_n": h_n[i * B_LOC : (i + 1) * B_LOC],
            "ident128": ident,
            "ones128": ones,
        }
        for i in range(N_CORES)
    ]
    res = run_bass_kernel_spmd(nc, in_maps, core_ids=list(range(N_CORES)))

    out = np.empty((B, 1, 2 * H), dtype=np.float32)
    for i in range(N_CORES):
        ctx = res.results[i]["ctx_out"]          # [B_LOC, H], unnormalized
        wv = res.results[i]["w_out"]             # [128, B_LOC * N_VT]
        lsum = (
            wv.astype(np.float64)
            .reshape(128, B_LOC, N_VT)
            .sum(axis=(0, 2))
        )                                        # [B_LOC]
        rows = slice(i * B_LOC, (i + 1) * B_LOC)
        out[rows, 0, :H] = h_n[rows]
        out[rows, 0, H:] = (ctx.astype(np.float64) / lsum[:, None]).astype(
            np.float32
        )
    return out
